# revision 17
# baseline (speedup 1.0000x reference)
"""ConvTranspose3d(64->32, k=3, stride=2, pad=1, out_pad=1, dilation=2) on 8 NeuronCores.

Math: with stride=2, dilation=2, padding=1, k=3, output position o = 2i + 2k - 1
is odd in every spatial dim, so the transposed conv collapses to a dense 3^3
conv y = conv3d(x, wc, padding=1) on the 32^3 grid (wc = flip(transpose(w))),
scattered into the odd sub-lattice of the 66^3 output; every other output
voxel is just bias. Verified exact vs the reference.

Sharding: 8 shards = 2 batches x 4 depth-blocks of 8 conv planes. Each core
computes its depth slab with an implicit GEMM: M = (c_out=32 x 4 depth planes)
on PSUM partitions, K = (64 c_in x 2 input planes) on SBUF partitions via a
block-Toeplitz-over-depth stationary operand (host-built), N = 512 hw pixels
per matmul, accumulating 27 matmuls (9 hw taps x 3 K-chunks) per PSUM bank.
float32r streams the PE at ~1 row/cycle for N>=256.
"""

import sys

sys.path.insert(0, "/opt/trn_rl_repo")

import numpy as np

N_CORES = 8
D_BLOCKS = 4  # depth blocks per batch
G_PER_CORE = 8  # conv output planes per core

_cache = {}


def _build_nc():
    import concourse.bass as bass
    import concourse.tile as tile
    from concourse import bacc, mybir

    dt = mybir.dt
    nc = bacc.Bacc("TRN2", target_bir_lowering=False, debug=False,
                   num_devices=N_CORES)

    # xs: 5 pairs of adjacent (zero-padded) input depth planes; partition
    # p = dpi*64 + ci. tw: 27 block-Toeplitz stationary matrices, columns
    # (chunk*9 + tap)*128 + (co*4 + gb). bias128: p = co*4+j -> bias[co].
    xs = nc.dram_tensor("xs", [5, 128, 34, 34], dt.float32r,
                        kind="ExternalInput")
    # w9[ci, u, t, co] = wc[co, ci, kd=5-u, t] (zeros for kd outside [0,3)):
    # compact staging from which the 27 block-Toeplitz stationary matrices
    # are assembled on-device with 6 strided DVE copies.
    w9 = nc.dram_tensor("w9", [64, 9 * 9 * 32], dt.float32,
                        kind="ExternalInput")
    bias = nc.dram_tensor("bias", [128, 1], dt.float32, kind="ExternalInput")
    # out planes indexed (q, parity): local output plane od = 2q + parity.
    out = nc.dram_tensor("out", [32, 8, 2, 66 * 66], dt.float32,
                         kind="ExternalOutput")

    with tile.TileContext(nc) as tc:
        with (
            tc.tile_pool(name="tw", bufs=1) as tw_pool,
            tc.tile_pool(name="xp", bufs=1) as xp_pool,
            tc.tile_pool(name="bias", bufs=1) as bias_pool,
            tc.tile_pool(name="bg", bufs=1) as bg_pool,
            tc.tile_pool(name="ot", bufs=1) as ot_pool,
            tc.tile_pool(name="ps", bufs=4, space="PSUM") as ps_pool,
        ):
            # Inputs, split so the first matmul group's dependencies
            # (weight staging + pair 0-2 lower rows) land first.
            tw_t = tw_pool.tile([128, 27 * 128], dt.float32r)
            w9_t = bias_pool.tile([64, 9 * 9 * 32], dt.float32, tag="w9")
            xp = []
            for p in range(5):
                xp_tile = xp_pool.tile([128, 34, 34], dt.float32r,
                                       tag=f"xp{p}")
                xp.append(xp_tile)

            bias_t = bias_pool.tile([128, 1], dt.float32)
            nc.sync.dma_start(bias_t[:], bias[:])
            nc.sync.dma_start(w9_t[:], w9[:])

            def load_xp(p, half, eng=None):
                rows = slice(0, 18) if half == 0 else slice(18, 34)
                (eng or nc.sync).dma_start(xp[p][:, rows, :],
                                           xs[p, :, rows, :])

            load_xp(0, 0)
            load_xp(1, 0)
            load_xp(2, 0)
            load_xp(0, 1)
            load_xp(1, 1)
            load_xp(2, 1)
            load_xp(3, 0)
            load_xp(4, 0)
            load_xp(3, 1)
            load_xp(4, 1)

            # assemble the block-Toeplitz stationary operand: for (c, dpi),
            # T[dpi*64+ci, (c,t,co,gb)] = w9[ci, (5-2c-dpi)+gb, t, co]
            tw_v = tw_t[:].rearrange("p (c t co gb) -> p c gb t co",
                                     c=3, t=9, co=32, gb=4)
            w9_v = w9_t[:].rearrange("q (u t co) -> q u t co", u=9, t=9)
            for c in range(3):
                for dpi in range(2):
                    u0 = 5 - 2 * c - dpi
                    nc.vector.tensor_copy(
                        tw_v[dpi * 64:(dpi + 1) * 64, c],
                        w9_v[:, u0:u0 + 4])

            # bias-broadcast plane-group tiles: partition p=(co*4+j), one
            # full 66x66 plane per partition, every element bias[co].
            bias_bc = bias_t[:].broadcast_to((128, 66 * 66))
            bg = bg_pool.tile([128, 66 * 66], dt.float32)
            nc.scalar.activation(bg[:], bias_bc,
                                 mybir.ActivationFunctionType.Copy)
            # even output planes (bias only): od = 2q, q in [0,8);
            # issued from ScalarE so Sync's issue slots stay on inputs
            nc.scalar.dma_start(out[:, 0:4, 0, :], bg[:])
            nc.scalar.dma_start(out[:, 4:8, 0, :], bg[:])
            ot = []
            for b in range(2):
                ot_b = ot_pool.tile([128, 66 * 66], dt.float32, tag=f"ot{b}")
                nc.scalar.activation(ot_b[:], bias_bc,
                                     mybir.ActivationFunctionType.Copy)
                ot.append(ot_b)

            prev_last_mm = None
            for b in range(2):
                ot_v = ot[b][:].rearrange("p (h a w c) -> p h a w c",
                                          h=33, a=2, c=2)
                ot_r = ot[b][:].rearrange("p (h w) -> p h w", h=66)
                for hh in range(2):
                    h0 = 16 * hh
                    ps = ps_pool.tile([128, 16, 32], dt.float32)
                    i = 0
                    for c in range(3):
                        src = xp[2 * b + c]
                        for t9 in range(9):
                            kh, kw = t9 // 3, t9 % 3
                            lhsT = tw_t[:, (c * 9 + t9) * 128:
                                        (c * 9 + t9 + 1) * 128]
                            rhs = src[:, h0 + kh:h0 + kh + 16, kw:kw + 32]
                            mm = nc.tensor.matmul(ps[:], lhsT, rhs,
                                                  start=(i == 0),
                                                  stop=(i == 26))
                            # keep the PE's static order group-contiguous so
                            # each scatter fires right after its 27th matmul
                            if i == 0 and prev_last_mm is not None:
                                tile.add_dep_helper(
                                    mm.ins, prev_last_mm.ins, sync=False,
                                    reason="group-contiguous PE order")
                            i += 1
                    prev_last_mm = mm
                    # scatter into odd (oh, ow) positions + bias add
                    dest = ot_v[:, h0:h0 + 16, 1, 0:32, 1]
                    nc.vector.tensor_scalar_add(dest, ps[:], bias_t[:])

                    # flush the finished half-plane rows of the odd planes
                    # (od = 2q+1, q = 4b+j) as soon as they are assembled
                    orows = slice(2 * h0, 2 * h0 + 32) if hh == 0 \
                        else slice(32, 66)
                    nc.scalar.dma_start(
                        out[:, 4 * b:4 * b + 4, 1, :]
                        .rearrange("c q (h w) -> c q h w", h=66)
                        [:, :, orows, :],
                        ot_r[:, orows, :])

    nc.compile()
    return nc


def _prep_shared(weight, bias):
    # wc[co, ci, kd, kh, kw] = weight[ci, co, 2-kd, 2-kh, 2-kw]
    wc = np.flip(np.transpose(weight, (1, 0, 2, 3, 4)), axis=(2, 3, 4))
    # w9[ci, u, t, co] = wc[co, ci, 5-u, t] for u in {3,4,5}, else 0
    w9 = np.zeros((64, 9, 9, 32), np.float32)
    for u in (3, 4, 5):
        w9[:, u] = wc[:, :, 5 - u].reshape(32, 64, 9).transpose(1, 2, 0)
    w9 = np.ascontiguousarray(w9.reshape(64, 9 * 9 * 32))
    bias128 = np.ascontiguousarray(
        np.repeat(bias.astype(np.float32), 4).reshape(128, 1))
    return w9, bias128


def _make_slab(x, n, cblk):
    # 5 pairs of spatially padded planes (34x34, zero border);
    # pair p = unpadded planes (8c-1+2p, 8c+2p)
    xs = np.zeros((5, 128, 34, 34), np.float32)
    lo = G_PER_CORE * cblk - 1
    for p in range(5):
        for dpi in range(2):
            d = lo + 2 * p + dpi
            if 0 <= d < 32:
                xs[p, dpi * 64:(dpi + 1) * 64, 1:33, 1:33] = x[n, :, d]
    return xs


def kernel(x, weight, bias):
    from concourse.bass_utils import run_bass_kernel_spmd

    if "nc" not in _cache:
        _cache["nc"] = _build_nc()
    nc = _cache["nc"]

    x = np.asarray(x, np.float32)
    weight = np.asarray(weight, np.float32)
    bias = np.asarray(bias, np.float32)

    w9, bias128 = _prep_shared(weight, bias)
    in_maps = []
    for core in range(N_CORES):
        n, cblk = divmod(core, D_BLOCKS)
        in_maps.append({"xs": _make_slab(x, n, cblk), "w9": w9,
                        "bias": bias128})

    res = run_bass_kernel_spmd(nc, in_maps, core_ids=list(range(N_CORES)))

    full = np.empty((2, 32, 66, 66, 66), np.float32)
    # trailing output_padding planes (od 64, 65) are pure bias
    full[:, :, 64:66] = bias[None, :, None, None, None]
    for core in range(N_CORES):
        n, cblk = divmod(core, D_BLOCKS)
        couts = res.results[core]["out"].reshape(32, 16, 66, 66)
        full[n, :, 16 * cblk:16 * cblk + 16] = couts
    return full


# revision 18
# speedup vs baseline: 1.0154x; 1.0154x over previous
"""ConvTranspose3d(64->32, k=3, stride=2, pad=1, out_pad=1, dilation=2) on 8 NeuronCores.

Math: with stride=2, dilation=2, padding=1, k=3, output position o = 2i + 2k - 1
is odd in every spatial dim, so the transposed conv collapses to a dense 3^3
conv y = conv3d(x, wc, padding=1) on the 32^3 grid (wc = flip(transpose(w))),
scattered into the odd sub-lattice of the 66^3 output; every other output
voxel is just bias. Verified exact vs the reference.

Sharding: 8 shards = 2 batches x 4 depth-blocks of 8 conv planes. Each core
computes its depth slab with an implicit GEMM: M = (c_out=32 x 4 depth planes)
on PSUM partitions, K = (64 c_in x 2 input planes) on SBUF partitions via a
block-Toeplitz-over-depth stationary operand (host-built), N = 512 hw pixels
per matmul, accumulating 27 matmuls (9 hw taps x 3 K-chunks) per PSUM bank.
float32r streams the PE at ~1 row/cycle for N>=256.
"""

import sys

sys.path.insert(0, "/opt/trn_rl_repo")

import numpy as np

N_CORES = 8
D_BLOCKS = 4  # depth blocks per batch
G_PER_CORE = 8  # conv output planes per core

_cache = {}


def _build_nc():
    import concourse.bass as bass
    import concourse.tile as tile
    from concourse import bacc, mybir

    dt = mybir.dt
    nc = bacc.Bacc("TRN2", target_bir_lowering=False, debug=False,
                   num_devices=N_CORES)

    # xs: 5 pairs of adjacent (zero-padded) input depth planes; partition
    # p = dpi*64 + ci. tw: 27 block-Toeplitz stationary matrices, columns
    # (chunk*9 + tap)*128 + (co*4 + gb). bias128: p = co*4+j -> bias[co].
    xs = nc.dram_tensor("xs", [5, 128, 34, 34], dt.float32r,
                        kind="ExternalInput")
    # w9[ci, u, t, co] = wc[co, ci, kd=5-u, t] (zeros for kd outside [0,3)):
    # compact staging from which the 27 block-Toeplitz stationary matrices
    # are assembled on-device with 6 strided DVE copies.
    w9 = nc.dram_tensor("w9", [64, 9 * 9 * 32], dt.float32,
                        kind="ExternalInput")
    bias = nc.dram_tensor("bias", [128, 1], dt.float32, kind="ExternalInput")
    # out planes indexed (q, parity): local output plane od = 2q + parity.
    out = nc.dram_tensor("out", [32, 8, 2, 66 * 66], dt.float32,
                         kind="ExternalOutput")

    with tile.TileContext(nc) as tc:
        with (
            tc.tile_pool(name="tw", bufs=1) as tw_pool,
            tc.tile_pool(name="xp", bufs=1) as xp_pool,
            tc.tile_pool(name="bias", bufs=1) as bias_pool,
            tc.tile_pool(name="bg", bufs=1) as bg_pool,
            tc.tile_pool(name="ot", bufs=1) as ot_pool,
            tc.tile_pool(name="ps", bufs=4, space="PSUM") as ps_pool,
        ):
            # Inputs, split so the first matmul group's dependencies
            # (weight staging + pair 0-2 lower rows) land first.
            tw_t = tw_pool.tile([128, 27 * 128], dt.float32r)
            w9_t = bias_pool.tile([64, 9 * 9 * 32], dt.float32, tag="w9")
            xp = []
            for p in range(5):
                xp_tile = xp_pool.tile([128, 34, 34], dt.float32r,
                                       tag=f"xp{p}")
                xp.append(xp_tile)

            bias_t = bias_pool.tile([128, 1], dt.float32)
            nc.sync.dma_start(bias_t[:], bias[:])

            def load_w9(ulo, uhi):
                nc.sync.dma_start(w9_t[:, ulo * 288:uhi * 288],
                                  w9[:, ulo * 288:uhi * 288])

            def load_xp(p, half, eng=None):
                rows = slice(0, 18) if half == 0 else slice(18, 34)
                (eng or nc.sync).dma_start(xp[p][:, rows, :],
                                           xs[p, :, rows, :])

            load_w9(4, 9)   # chunk 0 reads u in [4,9)
            load_xp(0, 0)
            load_w9(2, 4)   # chunk 1 adds u in [2,4)
            load_xp(1, 0)
            load_w9(0, 2)   # chunk 2 adds u in [0,2)
            load_xp(2, 0)
            load_xp(0, 1)
            load_xp(1, 1)
            load_xp(2, 1)
            load_xp(3, 0)
            load_xp(4, 0)
            load_xp(3, 1)
            load_xp(4, 1)

            # assemble the block-Toeplitz stationary operand: for (c, dpi),
            # T[dpi*64+ci, (c,t,co,gb)] = w9[ci, (5-2c-dpi)+gb, t, co]
            tw_v = tw_t[:].rearrange("p (c t co gb) -> p c gb t co",
                                     c=3, t=9, co=32, gb=4)
            w9_v = w9_t[:].rearrange("q (u t co) -> q u t co", u=9, t=9)
            for c in range(3):
                for dpi in range(2):
                    u0 = 5 - 2 * c - dpi
                    nc.vector.tensor_copy(
                        tw_v[dpi * 64:(dpi + 1) * 64, c],
                        w9_v[:, u0:u0 + 4])

            # bias-broadcast plane-group tiles: partition p=(co*4+j), one
            # full 66x66 plane per partition, every element bias[co].
            bias_bc = bias_t[:].broadcast_to((128, 66 * 66))
            bg = bg_pool.tile([128, 66 * 66], dt.float32)
            nc.scalar.activation(bg[:], bias_bc,
                                 mybir.ActivationFunctionType.Copy)
            # even output planes (bias only): od = 2q, q in [0,8);
            # issued from ScalarE so Sync's issue slots stay on inputs
            nc.scalar.dma_start(out[:, 0:4, 0, :], bg[:])
            nc.scalar.dma_start(out[:, 4:8, 0, :], bg[:])
            ot = []
            for b in range(2):
                ot_b = ot_pool.tile([128, 66 * 66], dt.float32, tag=f"ot{b}")
                nc.scalar.activation(ot_b[:], bias_bc,
                                     mybir.ActivationFunctionType.Copy)
                ot.append(ot_b)

            prev_last_mm = None
            for b in range(2):
                ot_v = ot[b][:].rearrange("p (h a w c) -> p h a w c",
                                          h=33, a=2, c=2)
                ot_r = ot[b][:].rearrange("p (h w) -> p h w", h=66)
                for hh in range(2):
                    h0 = 16 * hh
                    ps = ps_pool.tile([128, 16, 32], dt.float32)
                    i = 0
                    for c in range(3):
                        src = xp[2 * b + c]
                        for t9 in range(9):
                            kh, kw = t9 // 3, t9 % 3
                            lhsT = tw_t[:, (c * 9 + t9) * 128:
                                        (c * 9 + t9 + 1) * 128]
                            rhs = src[:, h0 + kh:h0 + kh + 16, kw:kw + 32]
                            mm = nc.tensor.matmul(ps[:], lhsT, rhs,
                                                  start=(i == 0),
                                                  stop=(i == 26))
                            # keep the PE's static order group-contiguous so
                            # each scatter fires right after its 27th matmul
                            if i == 0 and prev_last_mm is not None:
                                tile.add_dep_helper(
                                    mm.ins, prev_last_mm.ins, sync=False,
                                    reason="group-contiguous PE order")
                            i += 1
                    prev_last_mm = mm
                    # scatter into odd (oh, ow) positions + bias add
                    dest = ot_v[:, h0:h0 + 16, 1, 0:32, 1]
                    nc.vector.tensor_scalar_add(dest, ps[:], bias_t[:])

                    # flush the finished half-plane rows of the odd planes
                    # (od = 2q+1, q = 4b+j) as soon as they are assembled
                    orows = slice(2 * h0, 2 * h0 + 32) if hh == 0 \
                        else slice(32, 66)
                    nc.scalar.dma_start(
                        out[:, 4 * b:4 * b + 4, 1, :]
                        .rearrange("c q (h w) -> c q h w", h=66)
                        [:, :, orows, :],
                        ot_r[:, orows, :])

    nc.compile()
    return nc


def _prep_shared(weight, bias):
    # wc[co, ci, kd, kh, kw] = weight[ci, co, 2-kd, 2-kh, 2-kw]
    wc = np.flip(np.transpose(weight, (1, 0, 2, 3, 4)), axis=(2, 3, 4))
    # w9[ci, u, t, co] = wc[co, ci, 5-u, t] for u in {3,4,5}, else 0
    w9 = np.zeros((64, 9, 9, 32), np.float32)
    for u in (3, 4, 5):
        w9[:, u] = wc[:, :, 5 - u].reshape(32, 64, 9).transpose(1, 2, 0)
    w9 = np.ascontiguousarray(w9.reshape(64, 9 * 9 * 32))
    bias128 = np.ascontiguousarray(
        np.repeat(bias.astype(np.float32), 4).reshape(128, 1))
    return w9, bias128


def _make_slab(x, n, cblk):
    # 5 pairs of spatially padded planes (34x34, zero border);
    # pair p = unpadded planes (8c-1+2p, 8c+2p)
    xs = np.zeros((5, 128, 34, 34), np.float32)
    lo = G_PER_CORE * cblk - 1
    for p in range(5):
        for dpi in range(2):
            d = lo + 2 * p + dpi
            if 0 <= d < 32:
                xs[p, dpi * 64:(dpi + 1) * 64, 1:33, 1:33] = x[n, :, d]
    return xs


def kernel(x, weight, bias):
    from concourse.bass_utils import run_bass_kernel_spmd

    if "nc" not in _cache:
        _cache["nc"] = _build_nc()
    nc = _cache["nc"]

    x = np.asarray(x, np.float32)
    weight = np.asarray(weight, np.float32)
    bias = np.asarray(bias, np.float32)

    w9, bias128 = _prep_shared(weight, bias)
    in_maps = []
    for core in range(N_CORES):
        n, cblk = divmod(core, D_BLOCKS)
        in_maps.append({"xs": _make_slab(x, n, cblk), "w9": w9,
                        "bias": bias128})

    res = run_bass_kernel_spmd(nc, in_maps, core_ids=list(range(N_CORES)))

    full = np.empty((2, 32, 66, 66, 66), np.float32)
    # trailing output_padding planes (od 64, 65) are pure bias
    full[:, :, 64:66] = bias[None, :, None, None, None]
    for core in range(N_CORES):
        n, cblk = divmod(core, D_BLOCKS)
        couts = res.results[core]["out"].reshape(32, 16, 66, 66)
        full[n, :, 16 * cblk:16 * cblk + 16] = couts
    return full
